# revision 3
# baseline (speedup 1.0000x reference)
"""GuidedAttentionLoss on 8 Trainium2 NeuronCores (Bass/Tile).

loss = sum_b sum_{i<To_b, j<Ti_b} A[b,i,j] * (1 - exp(-(i - j*To_b/Ti_b)^2 / (2*sigma^2))) / B

Sharding: data-parallel over batch B=64 -> 8 batches per core.

Per-core device program (shapes hardcoded for B=64, T_out=2000, T_in=512):
  For each of 8 local batches x 16 i-blocks of 128 rows:
    - DMA A tile [128, 512] (i on partitions, j on free dim)
    - ACT:  t = Square(-u_b[j] + s*i)   where u_b[j] = s*c_b*j for j<Ti, BIG else
            (s = sqrt(1/(2 sigma^2)); so t = (i - c_b j)^2/(2 sigma^2), huge for j>=Ti)
    - ACT:  e = Exp(-t)    (= gaussian for j<Ti, exactly 0 for j>=Ti)
    - DVE:  tensor_tensor_reduce: sum_j A*e      -> racc2[:, col]
    - DVE:  tensor_tensor_reduce: sum_j A*maskJ  -> racc1[:, col]
  Epilogue: per-partition partial = sum_cols maskI * (racc1 - racc2), DMA out [128].
Host: loss = sum(partials over cores+partitions) / 64.  (i-mask maskI applied at
column granularity; j-mask folded into u/maskJ tables computed on host from lengths.)
"""

import os
import sys
from contextlib import ExitStack

import numpy as np

if "/opt/trn_rl_repo" not in sys.path:
    sys.path.insert(0, "/opt/trn_rl_repo")

B, T_OUT, T_IN = 64, 2000, 512
NCORES = 8
BPC = B // NCORES          # batches per core
P = 128                    # partitions
NKB = (T_OUT + P - 1) // P  # 16 i-blocks (last has 80 valid rows)
SIGMA = 0.4
S = float(np.sqrt(1.0 / (2.0 * SIGMA * SIGMA)))  # sqrt(3.125)
BIG = np.float32(1e19)     # (BIG)^2 = 1e38 < f32 max; exp(-1e38) == 0

_CACHE = {}


def _build_program():
    import concourse.mybir as mybir
    import concourse.tile as tile
    from concourse import bacc

    AF = mybir.ActivationFunctionType
    ALU = mybir.AluOpType
    F32 = mybir.dt.float32

    nc = bacc.Bacc(
        "TRN2",
        target_bir_lowering=False,
        debug=False,
        enable_asserts=False,
        num_devices=NCORES,
    )
    a_d = nc.dram_tensor("a", [BPC * T_OUT, T_IN], F32, kind="ExternalInput")
    u_d = nc.dram_tensor("urep", [P, BPC * T_IN], F32, kind="ExternalInput")
    mj_d = nc.dram_tensor("mjrep", [P, BPC * T_IN], F32, kind="ExternalInput")
    bk_d = nc.dram_tensor("biask", [P, NKB], F32, kind="ExternalInput")
    mi_d = nc.dram_tensor("maski", [P, BPC * NKB], F32, kind="ExternalInput")
    o_d = nc.dram_tensor("out", [P, 1], F32, kind="ExternalOutput")

    with ExitStack() as ctx:
        tc = ctx.enter_context(tile.TileContext(nc))
        const = ctx.enter_context(tc.tile_pool(name="const", bufs=1))
        apool = ctx.enter_context(tc.tile_pool(name="apool", bufs=4))
        tpool = ctx.enter_context(tc.tile_pool(name="tpool", bufs=3))
        epool = ctx.enter_context(tc.tile_pool(name="epool", bufs=3))
        qpool = ctx.enter_context(tc.tile_pool(name="qpool", bufs=2))

        u_s = const.tile([P, BPC * T_IN], F32)
        nc.sync.dma_start(u_s[:], u_d.ap())
        mj_s = const.tile([P, BPC * T_IN], F32)
        nc.sync.dma_start(mj_s[:], mj_d.ap())
        bk_s = const.tile([P, NKB], F32)
        nc.sync.dma_start(bk_s[:], bk_d.ap())
        mi_s = const.tile([P, BPC * NKB], F32)
        nc.sync.dma_start(mi_s[:], mi_d.ap())
        racc1 = const.tile([P, BPC * NKB], F32)
        racc2 = const.tile([P, BPC * NKB], F32)

        a_ap = a_d.ap()
        tail = T_OUT - (NKB - 1) * P  # 80 valid rows in the last block
        for b in range(BPC):
            for k in range(NKB):
                col = b * NKB + k
                at = apool.tile([P, T_IN], F32)
                r0 = b * T_OUT + k * P
                if k == NKB - 1:
                    # partition offsets must be 32-aligned: clear rows 64:128,
                    # then the DMA (traced after -> scheduled after) fills 0:80
                    nc.gpsimd.memset(at[64:P, :], 0.0)
                    nc.sync.dma_start(at[0:tail, :], a_ap[r0 : r0 + tail, :])
                else:
                    nc.sync.dma_start(at[:], a_ap[r0 : r0 + P, :])

                tt = tpool.tile([P, T_IN], F32)
                nc.scalar.activation(
                    tt[:],
                    u_s[:, b * T_IN : (b + 1) * T_IN],
                    AF.Square,
                    bias=bk_s[:, k : k + 1],
                    scale=-1.0,
                )
                et = epool.tile([P, T_IN], F32)
                nc.scalar.activation(et[:], tt[:], AF.Exp, scale=-1.0)

                # tensor_tensor_reduce crashes the exec unit on HW (probe2
                # bisection) -- use plain mul + reduce. One mul on GPSIMD to
                # offload the vector engine.
                q1 = qpool.tile([P, T_IN], F32, tag="q1")
                nc.vector.tensor_mul(q1[:], at[:], et[:])
                nc.vector.reduce_sum(
                    racc2[:, col : col + 1], q1[:], mybir.AxisListType.X
                )
                q2 = qpool.tile([P, T_IN], F32, tag="q2")
                nc.gpsimd.tensor_mul(
                    q2[:], at[:], mj_s[:, b * T_IN : (b + 1) * T_IN]
                )
                nc.vector.reduce_sum(
                    racc1[:, col : col + 1], q2[:], mybir.AxisListType.X
                )

        m = const.tile([P, BPC * NKB], F32)
        nc.vector.tensor_sub(m[:], racc1[:], racc2[:])
        m2 = const.tile([P, BPC * NKB], F32)
        nc.vector.tensor_mul(m2[:], m[:], mi_s[:])
        t2 = const.tile([P, 1], F32)
        nc.vector.reduce_sum(t2[:], m2[:], mybir.AxisListType.X)
        nc.sync.dma_start(o_d.ap(), t2[:])

    nc.compile()
    return nc


def _host_tables(input_lengths, output_lengths):
    """Per-core constant tables derived from the length vectors."""
    j = np.arange(T_IN, dtype=np.float64)
    i_of_pk = (np.arange(P, dtype=np.float64)[:, None]
               + P * np.arange(NKB, dtype=np.float64)[None, :])  # [128, 16]
    biask = (S * i_of_pk).astype(np.float32)

    per_core = []
    for c in range(NCORES):
        u_rows = np.empty((BPC, T_IN), np.float32)
        mj_rows = np.empty((BPC, T_IN), np.float32)
        mi = np.empty((P, BPC * NKB), np.float32)
        for b in range(BPC):
            gb = c * BPC + b
            Ti = float(input_lengths[gb])
            To = float(output_lengths[gb])
            cb = To / Ti
            u_rows[b] = np.where(j < Ti, S * cb * j, BIG).astype(np.float32)
            mj_rows[b] = (j < Ti).astype(np.float32)
            mi[:, b * NKB : (b + 1) * NKB] = (i_of_pk < To).astype(np.float32)
        urep = np.ascontiguousarray(
            np.broadcast_to(u_rows.reshape(1, BPC * T_IN), (P, BPC * T_IN))
        )
        mjrep = np.ascontiguousarray(
            np.broadcast_to(mj_rows.reshape(1, BPC * T_IN), (P, BPC * T_IN))
        )
        per_core.append({"urep": urep, "mjrep": mjrep, "biask": biask, "maski": mi})
    return per_core


last_results = None  # stashed BassKernelResults for test harness introspection


def kernel(alignments, input_lengths, output_lengths, **run_kwargs):
    global last_results
    from concourse import bass_utils

    alignments = np.ascontiguousarray(alignments, dtype=np.float32)
    input_lengths = np.asarray(input_lengths)
    output_lengths = np.asarray(output_lengths)
    assert alignments.shape == (B, T_OUT, T_IN)

    if "prog" not in _CACHE:
        _CACHE["prog"] = _build_program()
    nc = _CACHE["prog"]

    tables = _host_tables(input_lengths, output_lengths)
    in_maps = []
    for c in range(NCORES):
        a_shard = alignments[c * BPC : (c + 1) * BPC].reshape(BPC * T_OUT, T_IN)
        in_maps.append({"a": a_shard, **tables[c]})

    res = bass_utils.run_bass_kernel_spmd(
        nc, in_maps, core_ids=list(range(NCORES)), **run_kwargs
    )
    last_results = res

    total = np.float64(0.0)
    for c in range(NCORES):
        total += np.sum(res.results[c]["out"].astype(np.float64))
    return np.float32(total / B)
